# revision 90
# baseline (speedup 1.0000x reference)
"""Trainium2 Bass kernel for nn_Attention_45749991637079.

Reference computation (per batch b, C=192 channels, 128x128 image, 8 heads):
    qkv  = w_qkv @ x                       (1x1 conv; k-branch unused)
    q,v  = depthwise 3x3 (SAME) of the q/v channel blocks
    qd   = q[:, ::2, ::2]                  (64x64 downsample)
    attn = softmax(l2norm-rows(qd_h) gram * temp)   per head (24x24)
    out  = w_proj @ (attn @ v)             == Wf @ v_dw, Wf = Wp·blockdiag(A)

Sharding: data-parallel over batch; one batch per NeuronCore (8 cores).

Device algorithm per core (v2 layout):
  - Pointwise conv q+v MERGED into one M=384 pass (3 full 128-row M-chunks:
    M0 = q[0:128], M1 = q[128:192]|v[0:64], M2 = v[64:192]) so no matmul
    runs with M=64.
  - The 64-channel groups (q[128:192], v[0:64]) are re-laid out "split-half":
    partitions 0:64 hold the band's top-half image rows, partitions 64:128
    the bottom half, so the 9-tap depthwise diag-matmuls for them process
    half the columns at full 128-partition width.
  - Gram of downsampled q via DMA-transpose tiles + PSUM-accumulated
    matmuls per band; softmax / blockdiag / Wf fold as in the reference.
  - Final out = Wf @ v_dw as 512-col subtiles, output in fp16.
All weight transposes / diag-tap matrices are precomputed on host.
"""

import numpy as np

C = 192
H = W = 128
HW = H * W
HEADS = 8
CHD = 24
P0, P1 = 128, 64
BAND = 16                 # output image rows per band
NB = H // BAND            # 8 bands
PWR = BAND + 2            # pointwise rows computed per band (halo)
PBW = 130                 # padded row width (1 + 128 + 1)
PBH = PWR // 2 + 1        # split-half rows per half (10)
TAPS = [(di, dj) for di in range(3) for dj in range(3)]

_BUILT = {}


def _build(iters=1):
    import concourse.mybir as mybir
    import concourse.tile as tile
    from concourse import bacc

    f32 = mybir.dt.float32
    f16 = mybir.dt.float16
    Alu = mybir.AluOpType
    Act = mybir.ActivationFunctionType
    Ax = mybir.AxisListType

    nc = bacc.Bacc(
        "TRN2", target_bir_lowering=False, debug=False,
        enable_asserts=False, num_devices=8,
    )

    # DRAM I/O (per-core shapes)
    xb = nc.dram_tensor("xb", (C, HW), f16, kind="ExternalInput").ap()
    wk0 = nc.dram_tensor("wk0", (P0, 384), f16, kind="ExternalInput").ap()
    wk1 = nc.dram_tensor("wk1", (P1, 384), f16, kind="ExternalInput").ap()
    dq0 = nc.dram_tensor("dq0", (P0, 9 * P0), f16, kind="ExternalInput").ap()
    dq1s = nc.dram_tensor("dq1s", (P0, 9 * P0), f16, kind="ExternalInput").ap()
    dvA = nc.dram_tensor("dvA", (P0, 9 * P0), f16, kind="ExternalInput").ap()
    dv0s = nc.dram_tensor("dv0s", (P0, 9 * P0), f16, kind="ExternalInput").ap()
    wp = nc.dram_tensor("wp", (P0, 384), f32, kind="ExternalInput").ap()
    tq = nc.dram_tensor("tq", (C, 1), f32, kind="ExternalInput").ap()
    eye = nc.dram_tensor("eye", (P0, P0), f16, kind="ExternalInput").ap()
    dvAc = nc.dram_tensor("dvAc", (P0, 9), f16, kind="ExternalInput").ap()
    dv0c = nc.dram_tensor("dv0c", (P0, 9), f16, kind="ExternalInput").ap()
    out = nc.dram_tensor("out", (C, HW), f16, kind="ExternalOutput").ap()
    import os
    _dbg = os.environ.get("KDBG") == "1"
    if _dbg:
        dqd = nc.dram_tensor("dqd", (P0, 8192), f32, kind="ExternalOutput").ap()
        datt = nc.dram_tensor("datt", (CHD, C), f32, kind="ExternalOutput").ap()
        dvdw = nc.dram_tensor("dvdw", (P0, 3 * HW // 2), f16, kind="ExternalOutput").ap()

    import contextlib

    XBC = BAND * W  # x band cols per chunk (2048, no halo)

    with tile.TileContext(nc) as tc:
      with (tc.For_i(0, iters, 1) if iters > 1 else contextlib.nullcontext()):
        with (
            tc.tile_pool(name="const", bufs=1) as cp,
            tc.tile_pool(name="work", bufs=2) as wkp,
            tc.tile_pool(name="qdt", bufs=4) as qtp,
            tc.tile_pool(name="psA", bufs=2, space="PSUM") as psA,
            tc.tile_pool(name="psH", bufs=2, space="PSUM") as psH,
        ):
            # ---- constants ----
            wk0_sb = cp.tile([P0, 384], f16)
            wk1_sb = cp.tile([P1, 384], f16)
            dq0_sb = cp.tile([P0, 9 * P0], f16)
            dq1s_sb = cp.tile([P0, 9 * P0], f16)
            dvA_sb = cp.tile([P0, 9 * P0], f16)
            dv0s_sb = cp.tile([P0, 9 * P0], f16)
            wp_sb = cp.tile([P0, 384], f32)
            tq_sb = cp.tile([P0, 2], f32)
            eye_sb = cp.tile([P0, P0], f16)
            dvAc_sb = cp.tile([P0, 9], f16)
            dv0c_sb = cp.tile([P0, 9], f16)

            # big persistent buffers
            vdwA = cp.tile([P0, HW], f16)       # v chans 64:192
            vdwB = cp.tile([P0, HW // 2], f16)  # v chans 0:64, split-half
            qd0 = cp.tile([P0, 4096], f16)      # q chans 0:128, ds pixels
            qd1 = cp.tile([P1, 4096], f16)      # q chans 128:192
            g0a = cp.tile([P0, C], f32)
            g1a = cp.tile([P1, C], f32)
            srow = cp.tile([P0, C], f32)
            ssq0 = cp.tile([P0, NB], f32)
            ssq1s = cp.tile([P0, NB], f32)      # split-half: both halves
            att = cp.tile([CHD, C], f32)
            sm8 = cp.tile([CHD, 4 * HEADS], f32)
            rn = cp.tile([P0, 2], f32)
            scr = cp.tile([P0, 512], f32)
            A0 = cp.tile([P0, C], f32)
            A1 = cp.tile([P1, C], f32)
            wfA = cp.tile([P0, C], f16)         # WfT rows 64:192
            wfB = cp.tile([P0, C], f16)         # WfT rows 0:64, both halves

            # padded band buffers, 2 sets (manual double buffer)
            pbq0 = [cp.tile([P0, PWR * PBW], f16, name=f"pbq0_{i}")
                    for i in range(2)]
            pbvA = [cp.tile([P0, PWR * PBW], f16, name=f"pbvA_{i}")
                    for i in range(3)]
            pbq1 = [cp.tile([P0, PBH * PBW], f16, name=f"pbq1_{i}")
                    for i in range(2)]
            pbv0 = [cp.tile([P0, PBH * PBW], f16, name=f"pbv0_{i}")
                    for i in range(3)]

            # pw-critical constants first; everything else after the first
            # band's x DMA (issued in the band loop) so band 0 starts early
            nc.sync.dma_start(out=wk0_sb[:], in_=wk0[:])
            nc.sync.dma_start(out=wk1_sb[:], in_=wk1[:])

            def late_consts():
                nc.sync.dma_start(out=dq0_sb[:], in_=dq0[:])
                nc.sync.dma_start(out=dq1s_sb[:], in_=dq1s[:])
                nc.sync.dma_start(out=dvA_sb[:], in_=dvA[:])
                nc.sync.dma_start(out=dv0s_sb[:], in_=dv0s[:])
                nc.sync.dma_start(out=wp_sb[:, 0:192], in_=wp[:, 0:192])
                nc.sync.dma_start(out=wp_sb[0:P1, 192:384],
                                  in_=wp[0:P1, 192:384])
                nc.sync.dma_start(out=tq_sb[:, 0:1], in_=tq[0:P0, :])
                nc.sync.dma_start(out=tq_sb[0:P1, 1:2], in_=tq[P0:C, :])
                nc.sync.dma_start(out=eye_sb[:], in_=eye[:])
                nc.sync.dma_start(out=dvAc_sb[:], in_=dvAc[:])
                nc.sync.dma_start(out=dv0c_sb[:], in_=dv0c[:])

            # one-time pad-column zeroing for all pb buffers
            for buf in pbq0 + pbvA:
                v = buf[:].rearrange("p (r c) -> p r c", c=PBW)
                nc.gpsimd.memset(v[:, :, 0:1], 0.0)
                nc.gpsimd.memset(v[:, :, 129:130], 0.0)
            for buf in pbq1 + pbv0:
                v = buf[:].rearrange("p (r c) -> p r c", c=PBW)
                nc.gpsimd.memset(v[:, :, 0:1], 0.0)
                nc.gpsimd.memset(v[:, :, 129:130], 0.0)

            nc.gpsimd.memset(g0a[:], 0.0)
            nc.gpsimd.memset(g1a[:], 0.0)

            # PSUM->SBUF evacuation: only ACT and DVE may read PSUM
            def ecopy(idx, dst, src):
                if idx % 2 == 0:
                    nc.scalar.copy(dst, src)
                else:
                    nc.vector.tensor_copy(dst, src)

            def vset(b):
                return b % 2

            # taps offloaded from PE to the (otherwise idle) Pool engine,
            # accumulated in SBUF fp16 and merged during PSUM evacuation.
            # Pool supports only tensor_tensor/copy, so each tap is a
            # broadcast-multiply (+ add for the second tap).
            POOL_A = (0,)            # vA tap indices done on Pool
            POOL_B = (0,)            # v0 tap indices done on Pool

            DVE_A = (4,)             # vA tap done on DVE (fused mul-add)
            accs = {}

            def v_taps(b, offload=True, part="all"):
                """Depthwise taps of the v path for band b + vdw evacuation.
                part="early" excludes the groups whose pb rows include the
                next band's halo row (emitted later as part="late" so their
                skewed dependency doesn't block the PSUM tag rotation).
                Deferred for the last two bands so their PE work overlaps the
                attention-stats serial chain (offload=False there: PE has the
                idle window, engines are busy with the chain)."""
                h0 = b * BAND
                gA = {"all": range(4), "early": range(3), "late": (3,)}[part]
                gB = {"all": range(2), "early": range(1), "late": (1,)}[part]
                poolA = POOL_A if offload else ()
                poolB = POOL_B if offload else ()
                dveA = DVE_A if offload else ()
                vAv = pbvA[vset(b)][:].rearrange("p (r c) -> p r c", c=PBW)
                v0v = pbv0[vset(b)][:].rearrange("p (r c) -> p r c", c=PBW)
                accA, acc0 = accs.setdefault(b, (
                    wkp.tile([P0, 2048], f16, tag="accA", name=f"accA_{b}"),
                    wkp.tile([P0, 1024], f16, tag="acc0", name=f"acc0_{b}")))
                # per-group Pool ops so each group's accumulator is ready as
                # soon as its pb rows are, not after the whole band
                def pool_taps(pbv, acc, dcol, taps, g):
                    ga = acc[:, g * 512:(g + 1) * 512]
                    gav = ga.rearrange("p (r c) -> p r c", c=W)
                    for n, t in enumerate(taps):
                        di, dj = TAPS[t]
                        srcv = pbv[:, 4 * g + di:4 * g + di + 4, dj:dj + W]
                        wb = dcol[:, t:t + 1].unsqueeze(2).broadcast_to(
                            (P0, 4, W))
                        if n == 0:
                            nc.gpsimd.tensor_tensor(gav, srcv, wb, Alu.mult)
                        else:
                            tmpP = wkp.tile([P0, 512], f16, tag="ptmp")
                            tv = tmpP[:].rearrange("p (r c) -> p r c", c=W)
                            nc.gpsimd.tensor_tensor(tv, srcv, wb, Alu.mult)
                            nc.gpsimd.tensor_tensor(ga, ga, tmpP[:], Alu.add)

                for g in gA:
                    if poolA:
                        pool_taps(vAv, accA, dvAc_sb, poolA, g)
                    for t in dveA:
                        di, dj = TAPS[t]
                        ga = accA[:, g * 512:(g + 1) * 512]
                        nc.vector.scalar_tensor_tensor(
                            ga.rearrange("p (r c) -> p r c", c=W),
                            vAv[:, 4 * g + di:4 * g + di + 4, dj:dj + W],
                            dvAc_sb[:, t:t + 1],
                            ga.rearrange("p (r c) -> p r c", c=W),
                            Alu.mult, Alu.add)
                for g in gB:
                    if poolB:
                        pool_taps(v0v, acc0, dv0c_sb, poolB, g)
                for g in gA:
                    vt = psH.tile([P0, 512], f32, tag="tA", bufs=3)
                    o = vt[:].rearrange("p (r c) -> p r c", c=W)
                    pe_taps = [t for t in range(9)
                               if t not in poolA and t not in dveA]
                    for n, t in enumerate(pe_taps):
                        di, dj = TAPS[t]
                        nc.tensor.matmul(
                            o, dvA_sb[:, t * P0:(t + 1) * P0],
                            vAv[:, 4 * g + di:4 * g + di + 4, dj:dj + W],
                            start=(n == 0), stop=(n == len(pe_taps) - 1))
                    cs = (h0 + 4 * g) * W
                    if offload:
                        nc.vector.tensor_tensor(
                            vdwA[:, cs:cs + 512], vt[:],
                            accA[:, g * 512:(g + 1) * 512], Alu.add)
                    else:
                        ecopy(g, vdwA[:, cs:cs + 512], vt[:])
                for g in gB:
                    vt = psH.tile([P0, 512], f32, tag="tA", bufs=3)
                    o = vt[:].rearrange("p (r c) -> p r c", c=W)
                    pe_taps = [t for t in range(9) if t not in poolB]
                    for n, t in enumerate(pe_taps):
                        di, dj = TAPS[t]
                        nc.tensor.matmul(
                            o, dv0s_sb[:, t * P0:(t + 1) * P0],
                            v0v[:, 4 * g + di:4 * g + di + 4, dj:dj + W],
                            start=(n == 0), stop=(n == len(pe_taps) - 1))
                    cs = b * 1024 + g * 512
                    if offload:
                        nc.vector.tensor_tensor(
                            vdwB[:, cs:cs + 512], vt[:],
                            acc0[:, g * 512:(g + 1) * 512], Alu.add)
                    else:
                        ecopy(g, vdwB[:, cs:cs + 512], vt[:])

            # ========== band sweep ==========
            for b in range(NB):
                h0 = b * BAND
                xband = wkp.tile([P0, 2 * XBC], f16, tag="xband")
                nc.sync.dma_start(out=xband[:, 0:XBC],
                                  in_=xb[0:P0, h0 * W:(h0 + BAND) * W])
                nc.sync.dma_start(out=xband[0:P1, XBC:2 * XBC],
                                  in_=xb[P0:C, h0 * W:(h0 + BAND) * W])
                if b == 0:
                    late_consts()

                q0v = pbq0[b % 2][:].rearrange("p (r c) -> p r c", c=PBW)
                vAv = pbvA[vset(b)][:].rearrange("p (r c) -> p r c", c=PBW)
                q1v = pbq1[b % 2][:].rearrange("p (r c) -> p r c", c=PBW)
                v0v = pbv0[vset(b)][:].rearrange("p (r c) -> p r c", c=PBW)

                # zero halo rows at image edges (pw never writes them)
                if b == 0:
                    nc.gpsimd.memset(q0v[:, 0, :], 0.0)
                    nc.gpsimd.memset(vAv[:, 0, :], 0.0)
                    nc.gpsimd.memset(q1v[0:P1, 0, :], 0.0)
                    nc.gpsimd.memset(v0v[0:P1, 0, :], 0.0)
                if b == NB - 1:
                    nc.gpsimd.memset(q0v[:, PWR - 1, :], 0.0)
                    nc.gpsimd.memset(vAv[:, PWR - 1, :], 0.0)
                    nc.gpsimd.memset(q1v[P1:P0, PBH - 1, :], 0.0)
                    nc.gpsimd.memset(v0v[P1:P0, PBH - 1, :], 0.0)

                # ---- merged pointwise conv: 4 subtiles of 4 rows (N=512),
                # computing ONLY this band's 16 rows; boundary rows are also
                # copied into the neighbor bands' halo rows so no pw row is
                # ever recomputed ----
                for s in range(4):
                    lr = 4 * s + 1           # local pb row of first pw row
                    Pq0 = psA.tile([P0, 512], f32, tag="pw0")
                    Pmx = psA.tile([P0, 512], f32, tag="pw1")
                    PvA = psA.tile([P0, 512], f32, tag="pw2", bufs=1)
                    x0v = xband[:, s * 512:(s + 1) * 512]
                    x1v = xband[0:P1, XBC + s * 512:XBC + (s + 1) * 512]
                    for Pt, mlo in ((Pq0, 0), (Pmx, 128), (PvA, 256)):
                        nc.tensor.matmul(Pt[:], wk0_sb[:, mlo:mlo + 128],
                                         x0v, start=True, stop=False)
                        nc.tensor.matmul(Pt[:], wk1_sb[:, mlo:mlo + 128],
                                         x1v, start=False, stop=True)
                    pv0 = Pq0[:].rearrange("p (r c) -> p r c", c=W)
                    pvm = Pmx[:].rearrange("p (r c) -> p r c", c=W)
                    pvA_ = PvA[:].rearrange("p (r c) -> p r c", c=W)
                    nc.scalar.copy(q0v[:, lr:lr + 4, 1:129], pv0)
                    nc.vector.tensor_copy(vAv[:, lr:lr + 4, 1:129], pvA_)
                    # M1 split-half scatter (pw-local rows lr..lr+3; top half
                    # covers rows 0..9, bottom half rows 8..17)
                    t0, t1 = lr, min(lr + 4, PBH)
                    if t1 > t0:
                        nc.vector.tensor_copy(
                            q1v[0:P1, t0:t1, 1:129], pvm[0:P1, t0 - lr:t1 - lr, :])
                        nc.scalar.copy(
                            v0v[0:P1, t0:t1, 1:129], pvm[P1:P0, t0 - lr:t1 - lr, :])
                    b0, b1 = max(lr, PWR - PBH), lr + 4
                    if b1 > b0:
                        o = PWR - PBH
                        nc.vector.tensor_copy(
                            q1v[P1:P0, b0 - o:b1 - o, 1:129],
                            pvm[0:P1, b0 - lr:b1 - lr, :])
                        nc.scalar.copy(
                            v0v[P1:P0, b0 - o:b1 - o, 1:129],
                            pvm[P1:P0, b0 - lr:b1 - lr, :])
                    # cross-band halo copies
                    if s == 0 and b > 0:
                        pq = pbq0[(b - 1) % 2][:].rearrange(
                            "p (r c) -> p r c", c=PBW)
                        pA = pbvA[vset(b - 1)][:].rearrange(
                            "p (r c) -> p r c", c=PBW)
                        p1 = pbq1[(b - 1) % 2][:].rearrange(
                            "p (r c) -> p r c", c=PBW)
                        p0_ = pbv0[vset(b - 1)][:].rearrange(
                            "p (r c) -> p r c", c=PBW)
                        nc.scalar.copy(pq[:, PWR - 1, 1:129], pv0[:, 0, :])
                        nc.vector.tensor_copy(pA[:, PWR - 1, 1:129],
                                              pvA_[:, 0, :])
                        nc.vector.tensor_copy(p1[P1:P0, PBH - 1, 1:129],
                                              pvm[0:P1, 0, :])
                        nc.scalar.copy(p0_[P1:P0, PBH - 1, 1:129],
                                       pvm[P1:P0, 0, :])
                    if s == 3 and b < NB - 1:
                        pq = pbq0[(b + 1) % 2][:].rearrange(
                            "p (r c) -> p r c", c=PBW)
                        pA = pbvA[vset(b + 1)][:].rearrange(
                            "p (r c) -> p r c", c=PBW)
                        p1 = pbq1[(b + 1) % 2][:].rearrange(
                            "p (r c) -> p r c", c=PBW)
                        p0_ = pbv0[vset(b + 1)][:].rearrange(
                            "p (r c) -> p r c", c=PBW)
                        nc.scalar.copy(pq[:, 0, 1:129], pv0[:, 3, :])
                        nc.vector.tensor_copy(pA[:, 0, 1:129], pvA_[:, 3, :])
                        nc.vector.tensor_copy(p1[0:P1, 0, 1:129],
                                              pvm[0:P1, 3, :])
                        nc.scalar.copy(p0_[0:P1, 0, 1:129],
                                       pvm[P1:P0, 3, :])

                if b < NB - 2:
                    v_taps(b, part="early")

                # ---- q0 taps (downsampled, N=512) ----
                qt = psH.tile([P0, 512], f32, tag="tA", bufs=3)
                o = qt[:].rearrange("p (r c) -> p r c", c=64)
                for t, (di, dj) in enumerate(TAPS):
                    nc.tensor.matmul(
                        o, dq0_sb[:, t * P0:(t + 1) * P0],
                        q0v[:, di:di + BAND:2, dj:dj + W:2],
                        start=(t == 0), stop=(t == 8))
                nc.scalar.activation(scr[:], qt[:], Act.Square,
                                     accum_out=ssq0[:, b:b + 1])
                nc.vector.tensor_copy(qd0[:, b * 512:(b + 1) * 512], qt[:])

                # ---- q1 split-half taps (N=256) ----
                qt2 = psH.tile([P0, 512], f32, tag="tA", bufs=3)
                o = qt2[:, 0:256].rearrange("p (r c) -> p r c", c=64)
                for t, (di, dj) in enumerate(TAPS):
                    nc.tensor.matmul(
                        o, dq1s_sb[:, t * P0:(t + 1) * P0],
                        q1v[:, di:di + 8:2, dj:dj + W:2],
                        start=(t == 0), stop=(t == 8))
                nc.scalar.activation(scr[:, 0:256], qt2[:, 0:256], Act.Square,
                                     accum_out=ssq1s[:, b:b + 1])
                nc.scalar.copy(qd1[:, b * 512:b * 512 + 256],
                               qt2[0:P1, 0:256])
                nc.vector.tensor_copy(qd1[:, b * 512 + 256:b * 512 + 512],
                                      qt2[P1:P0, 0:256])

                # ---- gram contribution (PSUM-accumulated), deferred by
                # one band so the q-tap -> qd-copy -> transpose latency chain
                # is long-satisfied when it runs ----
                def gram(gb):
                    g0p = psH.tile([P0, 512], f32, tag="tA", bufs=3)
                    g1p = psH.tile([P0, 512], f32, tag="tA", bufs=3)
                    for kb in range(4):
                        c0 = gb * 512 + kb * 128
                        pt0 = psA.tile([P0, 1024], f16, tag="pw0")
                        pt1 = psA.tile([P0, 1024], f16, tag="pw1")
                        nc.tensor.transpose(pt0[:, 0:P0], qd0[:, c0:c0 + P0],
                                            eye_sb[:])
                        nc.tensor.transpose(pt1[:, 0:P1], qd1[0:P1, c0:c0 + P0],
                                            eye_sb[0:P1, 0:P1])
                        qdTt = qtp.tile([P0, C], f16, tag="qdT")
                        nc.scalar.copy(qdTt[:, 0:P0], pt0[:, 0:P0])
                        nc.vector.tensor_copy(qdTt[:, P0:C], pt1[:, 0:P1])
                        nc.tensor.matmul(g0p[:, 0:C], qdTt[:, 0:P0], qdTt[:],
                                         start=(kb == 0), stop=(kb == 3))
                        nc.tensor.matmul(g1p[0:P1, 0:C], qdTt[:, P0:C],
                                         qdTt[:],
                                         start=(kb == 0), stop=(kb == 3))
                    nc.vector.tensor_tensor(g0a[:], g0a[:], g0p[:, 0:C],
                                            Alu.add)
                    nc.vector.tensor_tensor(g1a[:], g1a[:], g1p[0:P1, 0:C],
                                            Alu.add)

                if b > 0:
                    gram(b - 1)
                if 0 < b < NB - 1:
                    v_taps(b - 1, part="late")

            gram(NB - 1)
            # deferred v-path taps of the last two bands: placed here so
            # the PSUM tag rotation doesn't chain them behind the attention
            # chain's tiles; their PE work fills the chain's latency bubble.
            v_taps(NB - 2, offload=False)
            v_taps(NB - 1, offload=False)

            # ---- row scales: rn = sqrt(temp) / ||qd_row|| ----
            nc.vector.tensor_copy(scr[0:P1, 4:4 + NB], ssq1s[P1:P0, :])
            nc.vector.tensor_tensor(ssq1s[0:P1, :], ssq1s[0:P1, :],
                                    scr[0:P1, 4:4 + NB], Alu.add)
            nc.vector.tensor_reduce(ssq0[:, 0:1], ssq0[:], Ax.X, Alu.add)
            nc.vector.tensor_reduce(ssq1s[0:P1, 0:1], ssq1s[0:P1, :],
                                    Ax.X, Alu.add)
            for ss_ap, rn_ap, tq_ap in (
                (ssq0[:, 0:1], rn[:, 0:1], tq_sb[:, 0:1]),
                (ssq1s[0:P1, 0:1], rn[0:P1, 1:2], tq_sb[0:P1, 1:2]),
            ):
                y = scr[0:ss_ap.shape[0], 0:1]
                yr = scr[0:ss_ap.shape[0], 1:2]
                nc.scalar.activation(y, ss_ap, Act.Sqrt)
                nc.vector.reciprocal(yr, y)
                nc.vector.tensor_tensor(yr, yr, ss_ap, Alu.mult)
                nc.vector.tensor_tensor(y, y, yr, Alu.add)
                nc.vector.tensor_scalar_mul(y, y, 0.5)
                nc.vector.reciprocal(rn_ap, y)
                nc.vector.tensor_tensor(rn_ap, rn_ap, tq_ap, Alu.mult)

            # attn = diag(s) G diag(s)
            nc.sync.dma_start(out=srow[0:1, 0:P0], in_=rn[:, 0:1])
            nc.sync.dma_start(out=srow[0:1, P0:C], in_=rn[0:P1, 1:2])
            nc.gpsimd.partition_broadcast(srow[:], srow[0:1, :])
            nc.vector.tensor_scalar_mul(g0a[:], g0a[:], rn[:, 0:1])
            nc.vector.tensor_scalar_mul(g1a[:], g1a[:], rn[0:P1, 1:2])
            nc.vector.tensor_tensor(g0a[:], g0a[:], srow[:], Alu.mult)
            nc.vector.tensor_tensor(g1a[:], g1a[:], srow[0:P1, :], Alu.mult)

            # ---- extract per-head diag blocks to compact (24, 192) ----
            for h in range(HEADS):
                eng = nc.sync
                c0 = h * CHD
                cs = slice(c0, c0 + CHD)
                dst = att[:, cs]
                if c0 + CHD <= P0:
                    eng.dma_start(out=dst, in_=g0a[cs, cs])
                elif c0 >= P0:
                    eng.dma_start(out=dst, in_=g1a[c0 - P0:c0 - P0 + CHD, cs])
                else:
                    n0 = P0 - c0
                    eng.dma_start(out=att[0:n0, cs], in_=g0a[c0:P0, cs])
                    eng.dma_start(out=att[n0:CHD, cs],
                                  in_=g1a[0:CHD - n0, cs])

            # ---- softmax over d within each head block ----
            attv = att[:].rearrange("p (h c) -> p h c", c=CHD)
            mx = sm8[:, 0:HEADS]
            nc.vector.tensor_reduce(mx, attv, Ax.X, Alu.max)
            nc.vector.tensor_tensor(attv, attv,
                                    mx.unsqueeze(2).broadcast_to((CHD, HEADS, CHD)),
                                    Alu.subtract)
            nc.scalar.activation(att[:], att[:], Act.Exp)
            sm = sm8[:, HEADS:2 * HEADS]
            nc.vector.tensor_reduce(sm, attv, Ax.X, Alu.add)
            rs = sm8[:, 2 * HEADS:3 * HEADS]
            nc.vector.reciprocal(rs, sm)
            nc.vector.tensor_tensor(attv, attv,
                                    rs.unsqueeze(2).broadcast_to((CHD, HEADS, CHD)),
                                    Alu.mult)
            if _dbg:
                nc.sync.dma_start(out=datt, in_=att[:])

            # ---- blockdiag(A) scatter + WfT = A_bd contraction with WpT ----
            nc.gpsimd.memset(A0[:], 0.0)
            nc.gpsimd.memset(A1[:], 0.0)
            for h in range(HEADS):
                eng = nc.sync
                c0 = h * CHD
                cs = slice(c0, c0 + CHD)
                srcb = att[:, cs]
                if c0 + CHD <= P0:
                    eng.dma_start(out=A0[cs, cs], in_=srcb)
                elif c0 >= P0:
                    eng.dma_start(out=A1[c0 - P0:c0 - P0 + CHD, cs], in_=srcb)
                else:
                    n0 = P0 - c0
                    eng.dma_start(out=A0[c0:P0, cs], in_=srcb[0:n0, :])
                    eng.dma_start(out=A1[0:CHD - n0, cs], in_=srcb[n0:CHD, :])
            pwf0 = psH.tile([P0, 512], f32, tag="tA", bufs=3)
            pwf1 = psH.tile([P0, 512], f32, tag="tA", bufs=3)
            nc.tensor.matmul(pwf0[:, 0:C], A0[:, 0:P0], wp_sb[:, 0:192],
                             start=True, stop=False)
            nc.tensor.matmul(pwf0[:, 0:C], A1[:, 0:P0], wp_sb[0:P1, 192:384],
                             start=False, stop=True)
            nc.tensor.matmul(pwf1[0:P1, 0:C], A0[:, P0:C], wp_sb[:, 0:192],
                             start=True, stop=False)
            nc.tensor.matmul(pwf1[0:P1, 0:C], A1[:, P0:C], wp_sb[0:P1, 192:384],
                             start=False, stop=True)
            # re-split WfT rows into (0:64, duplicated to both partition
            # halves for split-half rhs) and (64:192) chunks
            nc.scalar.copy(wfB[0:P1, :], pwf0[0:P1, 0:C])
            nc.vector.tensor_copy(wfB[P1:P0, :], pwf0[0:P1, 0:C])
            nc.vector.tensor_copy(wfA[0:P1, :], pwf0[P1:P0, 0:C])
            nc.scalar.copy(wfA[P1:P0, :], pwf1[0:P1, 0:C])

            if _dbg:
                nc.sync.dma_start(out=dqd[:, 0:4096], in_=qd0[:])
                nc.sync.dma_start(out=dqd[0:P1, 4096:8192], in_=qd1[:])
                nc.sync.dma_start(out=dvdw[:, 0:HW], in_=vdwA[:])
                nc.sync.dma_start(out=dvdw[:, HW:HW + HW // 2], in_=vdwB[:])

            # ========== final sweep: out = Wf-contraction @ v_dw ==========
            for i in range(HW // 512):
                po0 = psA.tile([P0, 512], f32, tag="pw0")
                po1 = psA.tile([P0, 512], f32, tag="pw1")
                rA = vdwA[:, i * 512:(i + 1) * 512]
                # split-half vdwB: band i//4, half (i%4)//2, group i%2
                hb = P1 * ((i % 4) // 2)
                csB = (i // 4) * 1024 + (i % 2) * 512
                rB = vdwB[hb:hb + P1, csB:csB + 512]
                wBv = wfB[hb:hb + P1, :]
                nc.tensor.matmul(po0[:], wfA[:, 0:P0], rA, start=True, stop=False)
                nc.tensor.matmul(po0[:], wBv[:, 0:P0], rB, start=False, stop=True)
                nc.tensor.matmul(po1[0:P1, :], wfA[:, P0:C], rA,
                                 start=True, stop=False)
                nc.tensor.matmul(po1[0:P1, :], wBv[:, P0:C], rB,
                                 start=False, stop=True)
                # flush staged output every 4 subtiles; final 4 split 2+2 so
                # the end-of-kernel drain waits on a smaller last DMA
                fb = [(k, min(k + 4, 28) if k < 28 else k + 2)
                      for k in (0, 4, 8, 12, 16, 20, 24, 28, 30)]
                grp = next((lo, hi) for lo, hi in fb if lo <= i < hi)
                lo, hi = grp
                j = i - lo
                if j == 0:
                    ost = wkp.tile([P0, 4096], f16, tag="ost", bufs=3)
                ecopy(i, ost[:, j * 512:(j + 1) * 512], po0[:])
                ecopy(i + 1, ost[0:P1, 2048 + j * 512:2048 + (j + 1) * 512],
                      po1[0:P1, :])
                if i == hi - 1:
                    nw = (hi - lo) * 512
                    nc.sync.dma_start(out=out[0:P0, lo * 512:hi * 512],
                                      in_=ost[:, 0:nw])
                    nc.sync.dma_start(out=out[P0:C, lo * 512:hi * 512],
                                      in_=ost[0:P1, 2048:2048 + nw])

    nc.compile()
    return nc


def _host_inputs(x, w_qkv, w_dw, w_proj, temperature):
    """Per-core input maps (host-side precompute of all weight transforms)."""
    f = np.float32
    h = np.float16
    W_q = w_qkv[0:C].astype(f)
    W_v = w_qkv[2 * C:3 * C].astype(f)
    wq_d = w_dw[0:C, 0].reshape(C, 9).astype(f)
    wv_d = w_dw[2 * C:3 * C, 0].reshape(C, 9).astype(f)
    WqT = W_q.T.astype(f)
    WvT = W_v.T.astype(f)

    wk0 = np.zeros((P0, 384), f)
    wk0[:, 0:128] = WqT[0:128, 0:128]
    wk0[:, 128:192] = WqT[0:128, 128:192]
    wk0[:, 192:256] = WvT[0:128, 0:64]
    wk0[:, 256:384] = WvT[0:128, 64:192]
    wk1 = np.zeros((P1, 384), f)
    wk1[:, 0:128] = WqT[128:192, 0:128]
    wk1[:, 128:192] = WqT[128:192, 128:192]
    wk1[:, 192:256] = WvT[128:192, 0:64]
    wk1[:, 256:384] = WvT[128:192, 64:192]

    def diag_pack(vals):
        # vals: (128, 9) per-partition diag values per tap
        outm = np.zeros((P0, 9 * P0), f)
        for t in range(9):
            np.fill_diagonal(outm[:, t * P0:(t + 1) * P0], vals[:, t])
        return outm

    idx_s = np.concatenate([np.arange(64), np.arange(64)])
    dq0 = diag_pack(wq_d[0:128])
    dq1s = diag_pack(wq_d[128 + idx_s])
    dvA = diag_pack(wv_d[64:192])
    dv0s = diag_pack(wv_d[idx_s])

    wp_pack = np.zeros((P0, 384), f)
    WpT = w_proj.T.astype(f)
    wp_pack[:, 0:192] = WpT[0:P0]
    wp_pack[0:P1, 192:384] = WpT[P0:C]

    tqv = np.sqrt(np.repeat(temperature.reshape(HEADS).astype(f), CHD)).reshape(C, 1)

    shared = {
        "wk0": wk0.astype(h), "wk1": wk1.astype(h),
        "dq0": dq0.astype(h), "dq1s": dq1s.astype(h),
        "dvA": dvA.astype(h), "dv0s": dv0s.astype(h),
        "wp": wp_pack, "tq": tqv, "eye": np.eye(P0, dtype=h),
        "dvAc": np.ascontiguousarray(wv_d[64:192]).astype(h),
        "dv0c": np.ascontiguousarray(wv_d[idx_s]).astype(h),
    }
    maps = []
    for b in range(8):
        m = dict(shared)
        m["xb"] = np.ascontiguousarray(x[b].reshape(C, HW).astype(h))
        maps.append(m)
    return maps


def kernel(x, w_qkv, w_dw, w_proj, temperature, _trace=False, _iters=1):
    from concourse.bass_utils import run_bass_kernel_spmd
    if _iters not in _BUILT:
        _BUILT[_iters] = _build(_iters)
    nc = _BUILT[_iters]
    in_maps = _host_inputs(
        np.asarray(x), np.asarray(w_qkv), np.asarray(w_dw),
        np.asarray(w_proj), np.asarray(temperature))
    res = run_bass_kernel_spmd(nc, in_maps, list(range(8)), trace=_trace)
    outs = [res.results[i]["out"].astype(np.float32).reshape(C, H, W)
            for i in range(8)]
    y = np.stack(outs, axis=0)
    kernel.last_result = res
    return y


# revision 95
# speedup vs baseline: 1.0004x; 1.0004x over previous
"""Trainium2 Bass kernel for nn_Attention_45749991637079.

Reference computation (per batch b, C=192 channels, 128x128 image, 8 heads):
    qkv  = w_qkv @ x                       (1x1 conv; k-branch unused)
    q,v  = depthwise 3x3 (SAME) of the q/v channel blocks
    qd   = q[:, ::2, ::2]                  (64x64 downsample)
    attn = softmax(l2norm-rows(qd_h) gram * temp)   per head (24x24)
    out  = w_proj @ (attn @ v)             == Wf @ v_dw, Wf = Wp·blockdiag(A)

Sharding: data-parallel over batch; one batch per NeuronCore (8 cores).

Device algorithm per core (v2 layout):
  - Pointwise conv q+v MERGED into one M=384 pass (3 full 128-row M-chunks:
    M0 = q[0:128], M1 = q[128:192]|v[0:64], M2 = v[64:192]) so no matmul
    runs with M=64.
  - The 64-channel groups (q[128:192], v[0:64]) are re-laid out "split-half":
    partitions 0:64 hold the band's top-half image rows, partitions 64:128
    the bottom half, so the 9-tap depthwise diag-matmuls for them process
    half the columns at full 128-partition width.
  - Gram of downsampled q via DMA-transpose tiles + PSUM-accumulated
    matmuls per band; softmax / blockdiag / Wf fold as in the reference.
  - Final out = Wf @ v_dw as 512-col subtiles, output in fp16.
All weight transposes / diag-tap matrices are precomputed on host.
"""

import numpy as np

C = 192
H = W = 128
HW = H * W
HEADS = 8
CHD = 24
P0, P1 = 128, 64
BAND = 16                 # output image rows per band
NB = H // BAND            # 8 bands
PWR = BAND + 2            # pointwise rows computed per band (halo)
PBW = 130                 # padded row width (1 + 128 + 1)
PBH = PWR // 2 + 1        # split-half rows per half (10)
TAPS = [(di, dj) for di in range(3) for dj in range(3)]

_BUILT = {}


def _build(iters=1):
    import concourse.mybir as mybir
    import concourse.tile as tile
    from concourse import bacc

    f32 = mybir.dt.float32
    f16 = mybir.dt.float16
    Alu = mybir.AluOpType
    Act = mybir.ActivationFunctionType
    Ax = mybir.AxisListType

    nc = bacc.Bacc(
        "TRN2", target_bir_lowering=False, debug=False,
        enable_asserts=False, num_devices=8,
    )

    # DRAM I/O (per-core shapes)
    xb = nc.dram_tensor("xb", (C, HW), f16, kind="ExternalInput").ap()
    wk0 = nc.dram_tensor("wk0", (P0, 384), f16, kind="ExternalInput").ap()
    wk1 = nc.dram_tensor("wk1", (P1, 384), f16, kind="ExternalInput").ap()
    dq0 = nc.dram_tensor("dq0", (P0, 9 * P0), f16, kind="ExternalInput").ap()
    dq1s = nc.dram_tensor("dq1s", (P0, 9 * P0), f16, kind="ExternalInput").ap()
    dvA = nc.dram_tensor("dvA", (P0, 9 * P0), f16, kind="ExternalInput").ap()
    dv0s = nc.dram_tensor("dv0s", (P0, 9 * P0), f16, kind="ExternalInput").ap()
    wp = nc.dram_tensor("wp", (P0, 384), f32, kind="ExternalInput").ap()
    tq = nc.dram_tensor("tq", (C, 1), f32, kind="ExternalInput").ap()
    eye = nc.dram_tensor("eye", (P0, P0), f16, kind="ExternalInput").ap()
    dvAc = nc.dram_tensor("dvAc", (P0, 9), f16, kind="ExternalInput").ap()
    dv0c = nc.dram_tensor("dv0c", (P0, 9), f16, kind="ExternalInput").ap()
    out = nc.dram_tensor("out", (C, HW), f16, kind="ExternalOutput").ap()
    import os
    _dbg = os.environ.get("KDBG") == "1"
    if _dbg:
        dqd = nc.dram_tensor("dqd", (P0, 8192), f32, kind="ExternalOutput").ap()
        datt = nc.dram_tensor("datt", (CHD, C), f32, kind="ExternalOutput").ap()
        dvdw = nc.dram_tensor("dvdw", (P0, 3 * HW // 2), f16, kind="ExternalOutput").ap()

    import contextlib

    XBC = BAND * W  # x band cols per chunk (2048, no halo)

    with tile.TileContext(nc) as tc:
      with (tc.For_i(0, iters, 1) if iters > 1 else contextlib.nullcontext()):
        with (
            tc.tile_pool(name="const", bufs=1) as cp,
            tc.tile_pool(name="work", bufs=2) as wkp,
            tc.tile_pool(name="qdt", bufs=6) as qtp,
            tc.tile_pool(name="psA", bufs=2, space="PSUM") as psA,
            tc.tile_pool(name="psH", bufs=2, space="PSUM") as psH,
        ):
            # ---- constants ----
            wk0_sb = cp.tile([P0, 384], f16)
            wk1_sb = cp.tile([P1, 384], f16)
            dq0_sb = cp.tile([P0, 9 * P0], f16)
            dq1s_sb = cp.tile([P0, 9 * P0], f16)
            dvA_sb = cp.tile([P0, 9 * P0], f16)
            dv0s_sb = cp.tile([P0, 9 * P0], f16)
            wp_sb = cp.tile([P0, 384], f32)
            tq_sb = cp.tile([P0, 2], f32)
            eye_sb = cp.tile([P0, P0], f16)
            dvAc_sb = cp.tile([P0, 9], f16)
            dv0c_sb = cp.tile([P0, 9], f16)

            # big persistent buffers
            vdwA = cp.tile([P0, HW], f16)       # v chans 64:192
            vdwB = cp.tile([P0, HW // 2], f16)  # v chans 0:64, split-half
            qd0 = cp.tile([P0, 4096], f16)      # q chans 0:128, ds pixels
            qd1 = cp.tile([P1, 4096], f16)      # q chans 128:192
            g0a = cp.tile([P0, C], f32)
            g1a = cp.tile([P1, C], f32)
            srow = cp.tile([P0, C], f32)
            ssq0 = cp.tile([P0, NB], f32)
            ssq1s = cp.tile([P0, NB], f32)      # split-half: both halves
            att = cp.tile([CHD, C], f32)
            sm8 = cp.tile([CHD, 4 * HEADS], f32)
            rn = cp.tile([P0, 2], f32)
            scr = cp.tile([P0, 512], f32)
            A0 = cp.tile([P0, C], f32)
            A1 = cp.tile([P1, C], f32)
            wfA = cp.tile([P0, C], f16)         # WfT rows 64:192
            wfB = cp.tile([P0, C], f16)         # WfT rows 0:64, both halves

            # padded band buffers, 2 sets (manual double buffer)
            pbq0 = [cp.tile([P0, PWR * PBW], f16, name=f"pbq0_{i}")
                    for i in range(2)]
            pbvA = [cp.tile([P0, PWR * PBW], f16, name=f"pbvA_{i}")
                    for i in range(3)]
            pbq1 = [cp.tile([P0, PBH * PBW], f16, name=f"pbq1_{i}")
                    for i in range(2)]
            pbv0 = [cp.tile([P0, PBH * PBW], f16, name=f"pbv0_{i}")
                    for i in range(3)]

            # pw-critical constants first; everything else after the first
            # band's x DMA (issued in the band loop) so band 0 starts early
            nc.sync.dma_start(out=wk0_sb[:], in_=wk0[:])
            nc.sync.dma_start(out=wk1_sb[:], in_=wk1[:])

            def late_consts():
                nc.sync.dma_start(out=dq0_sb[:], in_=dq0[:])
                nc.sync.dma_start(out=dq1s_sb[:], in_=dq1s[:])
                nc.sync.dma_start(out=dvA_sb[:], in_=dvA[:])
                nc.sync.dma_start(out=dv0s_sb[:], in_=dv0s[:])
                nc.sync.dma_start(out=wp_sb[:, 0:192], in_=wp[:, 0:192])
                nc.sync.dma_start(out=wp_sb[0:P1, 192:384],
                                  in_=wp[0:P1, 192:384])
                nc.sync.dma_start(out=tq_sb[:, 0:1], in_=tq[0:P0, :])
                nc.sync.dma_start(out=tq_sb[0:P1, 1:2], in_=tq[P0:C, :])
                nc.sync.dma_start(out=eye_sb[:], in_=eye[:])
                nc.sync.dma_start(out=dvAc_sb[:], in_=dvAc[:])
                nc.sync.dma_start(out=dv0c_sb[:], in_=dv0c[:])

            # one-time pad-column zeroing for all pb buffers
            for buf in pbq0 + pbvA:
                v = buf[:].rearrange("p (r c) -> p r c", c=PBW)
                nc.gpsimd.memset(v[:, :, 0:1], 0.0)
                nc.gpsimd.memset(v[:, :, 129:130], 0.0)
            for buf in pbq1 + pbv0:
                v = buf[:].rearrange("p (r c) -> p r c", c=PBW)
                nc.gpsimd.memset(v[:, :, 0:1], 0.0)
                nc.gpsimd.memset(v[:, :, 129:130], 0.0)

            nc.gpsimd.memset(g0a[:], 0.0)
            nc.gpsimd.memset(g1a[:], 0.0)

            # PSUM->SBUF evacuation: only ACT and DVE may read PSUM
            def ecopy(idx, dst, src):
                if idx % 2 == 0:
                    nc.scalar.copy(dst, src)
                else:
                    nc.vector.tensor_copy(dst, src)

            def vset(b):
                return b % 2

            # taps offloaded from PE to the (otherwise idle) Pool engine,
            # accumulated in SBUF fp16 and merged during PSUM evacuation.
            # Pool supports only tensor_tensor/copy, so each tap is a
            # broadcast-multiply (+ add for the second tap).
            POOL_A = (0,)            # vA tap indices done on Pool
            POOL_B = (0,)            # v0 tap indices done on Pool

            DVE_A = (4,)             # vA tap done on DVE (fused mul-add)
            accs = {}

            def v_taps(b, offload=True, part="all"):
                """Depthwise taps of the v path for band b + vdw evacuation.
                part="early" excludes the groups whose pb rows include the
                next band's halo row (emitted later as part="late" so their
                skewed dependency doesn't block the PSUM tag rotation).
                Deferred for the last two bands so their PE work overlaps the
                attention-stats serial chain (offload=False there: PE has the
                idle window, engines are busy with the chain)."""
                h0 = b * BAND
                gA = {"all": range(4), "early": range(3), "late": (3,)}[part]
                gB = {"all": range(2), "early": range(1), "late": (1,)}[part]
                poolA = POOL_A if offload else ()
                poolB = POOL_B if offload else ()
                dveA = DVE_A if offload else ()
                vAv = pbvA[vset(b)][:].rearrange("p (r c) -> p r c", c=PBW)
                v0v = pbv0[vset(b)][:].rearrange("p (r c) -> p r c", c=PBW)
                accA, acc0 = accs.setdefault(b, (
                    wkp.tile([P0, 2048], f16, tag="accA", name=f"accA_{b}"),
                    wkp.tile([P0, 1024], f16, tag="acc0", name=f"acc0_{b}")))
                # per-group Pool ops so each group's accumulator is ready as
                # soon as its pb rows are, not after the whole band
                def pool_taps(pbv, acc, dcol, taps, g):
                    ga = acc[:, g * 512:(g + 1) * 512]
                    gav = ga.rearrange("p (r c) -> p r c", c=W)
                    for n, t in enumerate(taps):
                        di, dj = TAPS[t]
                        srcv = pbv[:, 4 * g + di:4 * g + di + 4, dj:dj + W]
                        wb = dcol[:, t:t + 1].unsqueeze(2).broadcast_to(
                            (P0, 4, W))
                        if n == 0:
                            nc.gpsimd.tensor_tensor(gav, srcv, wb, Alu.mult)
                        else:
                            tmpP = wkp.tile([P0, 512], f16, tag="ptmp")
                            tv = tmpP[:].rearrange("p (r c) -> p r c", c=W)
                            nc.gpsimd.tensor_tensor(tv, srcv, wb, Alu.mult)
                            nc.gpsimd.tensor_tensor(ga, ga, tmpP[:], Alu.add)

                for g in gA:
                    if poolA:
                        pool_taps(vAv, accA, dvAc_sb, poolA, g)
                    for t in dveA:
                        di, dj = TAPS[t]
                        ga = accA[:, g * 512:(g + 1) * 512]
                        nc.vector.scalar_tensor_tensor(
                            ga.rearrange("p (r c) -> p r c", c=W),
                            vAv[:, 4 * g + di:4 * g + di + 4, dj:dj + W],
                            dvAc_sb[:, t:t + 1],
                            ga.rearrange("p (r c) -> p r c", c=W),
                            Alu.mult, Alu.add)
                for g in gB:
                    if poolB:
                        pool_taps(v0v, acc0, dv0c_sb, poolB, g)
                for g in gA:
                    vt = psH.tile([P0, 512], f32, tag="tA", bufs=3)
                    o = vt[:].rearrange("p (r c) -> p r c", c=W)
                    pe_taps = [t for t in range(9)
                               if t not in poolA and t not in dveA]
                    for n, t in enumerate(pe_taps):
                        di, dj = TAPS[t]
                        nc.tensor.matmul(
                            o, dvA_sb[:, t * P0:(t + 1) * P0],
                            vAv[:, 4 * g + di:4 * g + di + 4, dj:dj + W],
                            start=(n == 0), stop=(n == len(pe_taps) - 1))
                    cs = (h0 + 4 * g) * W
                    if offload:
                        nc.vector.tensor_tensor(
                            vdwA[:, cs:cs + 512], vt[:],
                            accA[:, g * 512:(g + 1) * 512], Alu.add)
                    else:
                        ecopy(g, vdwA[:, cs:cs + 512], vt[:])
                for g in gB:
                    vt = psH.tile([P0, 512], f32, tag="tA", bufs=3)
                    o = vt[:].rearrange("p (r c) -> p r c", c=W)
                    pe_taps = [t for t in range(9) if t not in poolB]
                    for n, t in enumerate(pe_taps):
                        di, dj = TAPS[t]
                        nc.tensor.matmul(
                            o, dv0s_sb[:, t * P0:(t + 1) * P0],
                            v0v[:, 4 * g + di:4 * g + di + 4, dj:dj + W],
                            start=(n == 0), stop=(n == len(pe_taps) - 1))
                    cs = b * 1024 + g * 512
                    if offload:
                        nc.vector.tensor_tensor(
                            vdwB[:, cs:cs + 512], vt[:],
                            acc0[:, g * 512:(g + 1) * 512], Alu.add)
                    else:
                        ecopy(g, vdwB[:, cs:cs + 512], vt[:])

            # ========== band sweep ==========
            for b in range(NB):
                h0 = b * BAND
                xband = wkp.tile([P0, 2 * XBC], f16, tag="xband")
                nc.sync.dma_start(out=xband[:, 0:XBC],
                                  in_=xb[0:P0, h0 * W:(h0 + BAND) * W])
                nc.sync.dma_start(out=xband[0:P1, XBC:2 * XBC],
                                  in_=xb[P0:C, h0 * W:(h0 + BAND) * W])
                if b == 0:
                    late_consts()

                q0v = pbq0[b % 2][:].rearrange("p (r c) -> p r c", c=PBW)
                vAv = pbvA[vset(b)][:].rearrange("p (r c) -> p r c", c=PBW)
                q1v = pbq1[b % 2][:].rearrange("p (r c) -> p r c", c=PBW)
                v0v = pbv0[vset(b)][:].rearrange("p (r c) -> p r c", c=PBW)

                # zero halo rows at image edges (pw never writes them)
                if b == 0:
                    nc.gpsimd.memset(q0v[:, 0, :], 0.0)
                    nc.gpsimd.memset(vAv[:, 0, :], 0.0)
                    nc.gpsimd.memset(q1v[0:P1, 0, :], 0.0)
                    nc.gpsimd.memset(v0v[0:P1, 0, :], 0.0)
                if b == NB - 1:
                    nc.gpsimd.memset(q0v[:, PWR - 1, :], 0.0)
                    nc.gpsimd.memset(vAv[:, PWR - 1, :], 0.0)
                    nc.gpsimd.memset(q1v[P1:P0, PBH - 1, :], 0.0)
                    nc.gpsimd.memset(v0v[P1:P0, PBH - 1, :], 0.0)

                # ---- merged pointwise conv: 4 subtiles of 4 rows (N=512),
                # computing ONLY this band's 16 rows; boundary rows are also
                # copied into the neighbor bands' halo rows so no pw row is
                # ever recomputed ----
                for s in range(4):
                    lr = 4 * s + 1           # local pb row of first pw row
                    Pq0 = psA.tile([P0, 512], f32, tag="pw0")
                    Pmx = psA.tile([P0, 512], f32, tag="pw1")
                    PvA = psA.tile([P0, 512], f32, tag="pw2", bufs=1)
                    x0v = xband[:, s * 512:(s + 1) * 512]
                    x1v = xband[0:P1, XBC + s * 512:XBC + (s + 1) * 512]
                    for Pt, mlo in ((Pq0, 0), (Pmx, 128), (PvA, 256)):
                        nc.tensor.matmul(Pt[:], wk0_sb[:, mlo:mlo + 128],
                                         x0v, start=True, stop=False)
                        nc.tensor.matmul(Pt[:], wk1_sb[:, mlo:mlo + 128],
                                         x1v, start=False, stop=True)
                    pv0 = Pq0[:].rearrange("p (r c) -> p r c", c=W)
                    pvm = Pmx[:].rearrange("p (r c) -> p r c", c=W)
                    pvA_ = PvA[:].rearrange("p (r c) -> p r c", c=W)
                    nc.scalar.copy(q0v[:, lr:lr + 4, 1:129], pv0)
                    nc.vector.tensor_copy(vAv[:, lr:lr + 4, 1:129], pvA_)
                    # M1 split-half scatter (pw-local rows lr..lr+3; top half
                    # covers rows 0..9, bottom half rows 8..17)
                    t0, t1 = lr, min(lr + 4, PBH)
                    if t1 > t0:
                        nc.vector.tensor_copy(
                            q1v[0:P1, t0:t1, 1:129], pvm[0:P1, t0 - lr:t1 - lr, :])
                        nc.scalar.copy(
                            v0v[0:P1, t0:t1, 1:129], pvm[P1:P0, t0 - lr:t1 - lr, :])
                    b0, b1 = max(lr, PWR - PBH), lr + 4
                    if b1 > b0:
                        o = PWR - PBH
                        nc.vector.tensor_copy(
                            q1v[P1:P0, b0 - o:b1 - o, 1:129],
                            pvm[0:P1, b0 - lr:b1 - lr, :])
                        nc.scalar.copy(
                            v0v[P1:P0, b0 - o:b1 - o, 1:129],
                            pvm[P1:P0, b0 - lr:b1 - lr, :])
                    # cross-band halo copies
                    if s == 0 and b > 0:
                        pq = pbq0[(b - 1) % 2][:].rearrange(
                            "p (r c) -> p r c", c=PBW)
                        pA = pbvA[vset(b - 1)][:].rearrange(
                            "p (r c) -> p r c", c=PBW)
                        p1 = pbq1[(b - 1) % 2][:].rearrange(
                            "p (r c) -> p r c", c=PBW)
                        p0_ = pbv0[vset(b - 1)][:].rearrange(
                            "p (r c) -> p r c", c=PBW)
                        nc.scalar.copy(pq[:, PWR - 1, 1:129], pv0[:, 0, :])
                        nc.vector.tensor_copy(pA[:, PWR - 1, 1:129],
                                              pvA_[:, 0, :])
                        nc.vector.tensor_copy(p1[P1:P0, PBH - 1, 1:129],
                                              pvm[0:P1, 0, :])
                        nc.scalar.copy(p0_[P1:P0, PBH - 1, 1:129],
                                       pvm[P1:P0, 0, :])
                    if s == 3 and b < NB - 1:
                        pq = pbq0[(b + 1) % 2][:].rearrange(
                            "p (r c) -> p r c", c=PBW)
                        pA = pbvA[vset(b + 1)][:].rearrange(
                            "p (r c) -> p r c", c=PBW)
                        p1 = pbq1[(b + 1) % 2][:].rearrange(
                            "p (r c) -> p r c", c=PBW)
                        p0_ = pbv0[vset(b + 1)][:].rearrange(
                            "p (r c) -> p r c", c=PBW)
                        nc.scalar.copy(pq[:, 0, 1:129], pv0[:, 3, :])
                        nc.vector.tensor_copy(pA[:, 0, 1:129], pvA_[:, 3, :])
                        nc.vector.tensor_copy(p1[0:P1, 0, 1:129],
                                              pvm[0:P1, 3, :])
                        nc.scalar.copy(p0_[0:P1, 0, 1:129],
                                       pvm[P1:P0, 3, :])

                if b < NB - 2:
                    v_taps(b, part="early")

                # ---- q0 taps (downsampled, N=512) ----
                qt = psH.tile([P0, 512], f32, tag="tA", bufs=3)
                o = qt[:].rearrange("p (r c) -> p r c", c=64)
                for t, (di, dj) in enumerate(TAPS):
                    nc.tensor.matmul(
                        o, dq0_sb[:, t * P0:(t + 1) * P0],
                        q0v[:, di:di + BAND:2, dj:dj + W:2],
                        start=(t == 0), stop=(t == 8))
                nc.scalar.activation(scr[:], qt[:], Act.Square,
                                     accum_out=ssq0[:, b:b + 1])
                nc.vector.tensor_copy(qd0[:, b * 512:(b + 1) * 512], qt[:])

                # ---- q1 split-half taps (N=256) ----
                qt2 = psH.tile([P0, 512], f32, tag="tA", bufs=3)
                o = qt2[:, 0:256].rearrange("p (r c) -> p r c", c=64)
                for t, (di, dj) in enumerate(TAPS):
                    nc.tensor.matmul(
                        o, dq1s_sb[:, t * P0:(t + 1) * P0],
                        q1v[:, di:di + 8:2, dj:dj + W:2],
                        start=(t == 0), stop=(t == 8))
                nc.scalar.activation(scr[:, 0:256], qt2[:, 0:256], Act.Square,
                                     accum_out=ssq1s[:, b:b + 1])
                nc.scalar.copy(qd1[:, b * 512:b * 512 + 256],
                               qt2[0:P1, 0:256])
                nc.vector.tensor_copy(qd1[:, b * 512 + 256:b * 512 + 512],
                                      qt2[P1:P0, 0:256])

                # ---- gram contribution (PSUM-accumulated), deferred by
                # one band so the q-tap -> qd-copy -> transpose latency chain
                # is long-satisfied when it runs ----
                def gram(gb):
                    g0p = psH.tile([P0, 512], f32, tag="tA", bufs=3)
                    g1p = psH.tile([P0, 512], f32, tag="tA", bufs=3)
                    for kb in range(4):
                        c0 = gb * 512 + kb * 128
                        pt0 = psA.tile([P0, 1024], f16, tag="pw0")
                        pt1 = psA.tile([P0, 1024], f16, tag="pw1")
                        nc.tensor.transpose(pt0[:, 0:P0], qd0[:, c0:c0 + P0],
                                            eye_sb[:])
                        nc.tensor.transpose(pt1[:, 0:P1], qd1[0:P1, c0:c0 + P0],
                                            eye_sb[0:P1, 0:P1])
                        qdTt = qtp.tile([P0, C], f16, tag="qdT")
                        nc.scalar.copy(qdTt[:, 0:P0], pt0[:, 0:P0])
                        nc.vector.tensor_copy(qdTt[:, P0:C], pt1[:, 0:P1])
                        nc.tensor.matmul(g0p[:, 0:C], qdTt[:, 0:P0], qdTt[:],
                                         start=(kb == 0), stop=(kb == 3))
                        nc.tensor.matmul(g1p[0:P1, 0:C], qdTt[:, P0:C],
                                         qdTt[:],
                                         start=(kb == 0), stop=(kb == 3))
                    nc.vector.tensor_tensor(g0a[:], g0a[:], g0p[:, 0:C],
                                            Alu.add)
                    nc.vector.tensor_tensor(g1a[:], g1a[:], g1p[0:P1, 0:C],
                                            Alu.add)

                if b > 0:
                    gram(b - 1)
                if 0 < b < NB - 1:
                    v_taps(b - 1, part="late")

            gram(NB - 1)
            # deferred v-path taps of the last two bands: placed here so
            # the PSUM tag rotation doesn't chain them behind the attention
            # chain's tiles; their PE work fills the chain's latency bubble.
            v_taps(NB - 2, offload=False)
            v_taps(NB - 1, offload=False)

            # ---- row scales: rn = sqrt(temp) / ||qd_row|| ----
            nc.vector.tensor_copy(scr[0:P1, 4:4 + NB], ssq1s[P1:P0, :])
            nc.vector.tensor_tensor(ssq1s[0:P1, :], ssq1s[0:P1, :],
                                    scr[0:P1, 4:4 + NB], Alu.add)
            nc.vector.tensor_reduce(ssq0[:, 0:1], ssq0[:], Ax.X, Alu.add)
            nc.vector.tensor_reduce(ssq1s[0:P1, 0:1], ssq1s[0:P1, :],
                                    Ax.X, Alu.add)
            for ss_ap, rn_ap, tq_ap in (
                (ssq0[:, 0:1], rn[:, 0:1], tq_sb[:, 0:1]),
                (ssq1s[0:P1, 0:1], rn[0:P1, 1:2], tq_sb[0:P1, 1:2]),
            ):
                y = scr[0:ss_ap.shape[0], 0:1]
                yr = scr[0:ss_ap.shape[0], 1:2]
                nc.scalar.activation(y, ss_ap, Act.Sqrt)
                nc.vector.reciprocal(yr, y)
                nc.vector.tensor_tensor(yr, yr, ss_ap, Alu.mult)
                nc.vector.tensor_tensor(y, y, yr, Alu.add)
                nc.vector.tensor_scalar_mul(y, y, 0.5)
                nc.vector.reciprocal(rn_ap, y)
                nc.vector.tensor_tensor(rn_ap, rn_ap, tq_ap, Alu.mult)

            # attn = diag(s) G diag(s)
            nc.sync.dma_start(out=srow[0:1, 0:P0], in_=rn[:, 0:1])
            nc.sync.dma_start(out=srow[0:1, P0:C], in_=rn[0:P1, 1:2])
            nc.gpsimd.partition_broadcast(srow[:], srow[0:1, :])
            nc.vector.tensor_scalar_mul(g0a[:], g0a[:], rn[:, 0:1])
            nc.vector.tensor_scalar_mul(g1a[:], g1a[:], rn[0:P1, 1:2])
            nc.vector.tensor_tensor(g0a[:], g0a[:], srow[:], Alu.mult)
            nc.vector.tensor_tensor(g1a[:], g1a[:], srow[0:P1, :], Alu.mult)

            # ---- extract per-head diag blocks to compact (24, 192) ----
            for h in range(HEADS):
                eng = nc.sync
                c0 = h * CHD
                cs = slice(c0, c0 + CHD)
                dst = att[:, cs]
                if c0 + CHD <= P0:
                    eng.dma_start(out=dst, in_=g0a[cs, cs])
                elif c0 >= P0:
                    eng.dma_start(out=dst, in_=g1a[c0 - P0:c0 - P0 + CHD, cs])
                else:
                    n0 = P0 - c0
                    eng.dma_start(out=att[0:n0, cs], in_=g0a[c0:P0, cs])
                    eng.dma_start(out=att[n0:CHD, cs],
                                  in_=g1a[0:CHD - n0, cs])

            # ---- softmax over d within each head block ----
            attv = att[:].rearrange("p (h c) -> p h c", c=CHD)
            mx = sm8[:, 0:HEADS]
            nc.vector.tensor_reduce(mx, attv, Ax.X, Alu.max)
            nc.vector.tensor_tensor(attv, attv,
                                    mx.unsqueeze(2).broadcast_to((CHD, HEADS, CHD)),
                                    Alu.subtract)
            nc.scalar.activation(att[:], att[:], Act.Exp)
            sm = sm8[:, HEADS:2 * HEADS]
            nc.vector.tensor_reduce(sm, attv, Ax.X, Alu.add)
            rs = sm8[:, 2 * HEADS:3 * HEADS]
            nc.vector.reciprocal(rs, sm)
            nc.vector.tensor_tensor(attv, attv,
                                    rs.unsqueeze(2).broadcast_to((CHD, HEADS, CHD)),
                                    Alu.mult)
            if _dbg:
                nc.sync.dma_start(out=datt, in_=att[:])

            # ---- blockdiag(A) scatter + WfT = A_bd contraction with WpT ----
            nc.gpsimd.memset(A0[:], 0.0)
            nc.gpsimd.memset(A1[:], 0.0)
            for h in range(HEADS):
                eng = nc.sync
                c0 = h * CHD
                cs = slice(c0, c0 + CHD)
                srcb = att[:, cs]
                if c0 + CHD <= P0:
                    eng.dma_start(out=A0[cs, cs], in_=srcb)
                elif c0 >= P0:
                    eng.dma_start(out=A1[c0 - P0:c0 - P0 + CHD, cs], in_=srcb)
                else:
                    n0 = P0 - c0
                    eng.dma_start(out=A0[c0:P0, cs], in_=srcb[0:n0, :])
                    eng.dma_start(out=A1[0:CHD - n0, cs], in_=srcb[n0:CHD, :])
            pwf0 = psH.tile([P0, 512], f32, tag="tA", bufs=3)
            pwf1 = psH.tile([P0, 512], f32, tag="tA", bufs=3)
            nc.tensor.matmul(pwf0[:, 0:C], A0[:, 0:P0], wp_sb[:, 0:192],
                             start=True, stop=False)
            nc.tensor.matmul(pwf0[:, 0:C], A1[:, 0:P0], wp_sb[0:P1, 192:384],
                             start=False, stop=True)
            nc.tensor.matmul(pwf1[0:P1, 0:C], A0[:, P0:C], wp_sb[:, 0:192],
                             start=True, stop=False)
            nc.tensor.matmul(pwf1[0:P1, 0:C], A1[:, P0:C], wp_sb[0:P1, 192:384],
                             start=False, stop=True)
            # re-split WfT rows into (0:64, duplicated to both partition
            # halves for split-half rhs) and (64:192) chunks
            nc.scalar.copy(wfB[0:P1, :], pwf0[0:P1, 0:C])
            nc.vector.tensor_copy(wfB[P1:P0, :], pwf0[0:P1, 0:C])
            nc.vector.tensor_copy(wfA[0:P1, :], pwf0[P1:P0, 0:C])
            nc.scalar.copy(wfA[P1:P0, :], pwf1[0:P1, 0:C])

            if _dbg:
                nc.sync.dma_start(out=dqd[:, 0:4096], in_=qd0[:])
                nc.sync.dma_start(out=dqd[0:P1, 4096:8192], in_=qd1[:])
                nc.sync.dma_start(out=dvdw[:, 0:HW], in_=vdwA[:])
                nc.sync.dma_start(out=dvdw[:, HW:HW + HW // 2], in_=vdwB[:])

            # ========== final sweep: out = Wf-contraction @ v_dw ==========
            for i in range(HW // 512):
                po0 = psA.tile([P0, 512], f32, tag="pw0")
                po1 = psA.tile([P0, 512], f32, tag="pw1")
                rA = vdwA[:, i * 512:(i + 1) * 512]
                # split-half vdwB: band i//4, half (i%4)//2, group i%2
                hb = P1 * ((i % 4) // 2)
                csB = (i // 4) * 1024 + (i % 2) * 512
                rB = vdwB[hb:hb + P1, csB:csB + 512]
                wBv = wfB[hb:hb + P1, :]
                nc.tensor.matmul(po0[:], wfA[:, 0:P0], rA, start=True, stop=False)
                nc.tensor.matmul(po0[:], wBv[:, 0:P0], rB, start=False, stop=True)
                nc.tensor.matmul(po1[0:P1, :], wfA[:, P0:C], rA,
                                 start=True, stop=False)
                nc.tensor.matmul(po1[0:P1, :], wBv[:, P0:C], rB,
                                 start=False, stop=True)
                # flush staged output every 4 subtiles; final 4 split 2+2 so
                # the end-of-kernel drain waits on a smaller last DMA
                fb = [(k, min(k + 4, 28) if k < 28 else k + 2)
                      for k in (0, 4, 8, 12, 16, 20, 24, 28, 30)]
                grp = next((lo, hi) for lo, hi in fb if lo <= i < hi)
                lo, hi = grp
                j = i - lo
                if j == 0:
                    ost = wkp.tile([P0, 4096], f16, tag="ost", bufs=3)
                ecopy(i, ost[:, j * 512:(j + 1) * 512], po0[:])
                ecopy(i + 1, ost[0:P1, 2048 + j * 512:2048 + (j + 1) * 512],
                      po1[0:P1, :])
                if i == hi - 1:
                    nw = (hi - lo) * 512
                    nc.sync.dma_start(out=out[0:P0, lo * 512:hi * 512],
                                      in_=ost[:, 0:nw])
                    nc.sync.dma_start(out=out[P0:C, lo * 512:hi * 512],
                                      in_=ost[0:P1, 2048:2048 + nw])

    nc.compile()
    return nc


def _host_inputs(x, w_qkv, w_dw, w_proj, temperature):
    """Per-core input maps (host-side precompute of all weight transforms)."""
    f = np.float32
    h = np.float16
    W_q = w_qkv[0:C].astype(f)
    W_v = w_qkv[2 * C:3 * C].astype(f)
    wq_d = w_dw[0:C, 0].reshape(C, 9).astype(f)
    wv_d = w_dw[2 * C:3 * C, 0].reshape(C, 9).astype(f)
    WqT = W_q.T.astype(f)
    WvT = W_v.T.astype(f)

    wk0 = np.zeros((P0, 384), f)
    wk0[:, 0:128] = WqT[0:128, 0:128]
    wk0[:, 128:192] = WqT[0:128, 128:192]
    wk0[:, 192:256] = WvT[0:128, 0:64]
    wk0[:, 256:384] = WvT[0:128, 64:192]
    wk1 = np.zeros((P1, 384), f)
    wk1[:, 0:128] = WqT[128:192, 0:128]
    wk1[:, 128:192] = WqT[128:192, 128:192]
    wk1[:, 192:256] = WvT[128:192, 0:64]
    wk1[:, 256:384] = WvT[128:192, 64:192]

    def diag_pack(vals):
        # vals: (128, 9) per-partition diag values per tap
        outm = np.zeros((P0, 9 * P0), f)
        for t in range(9):
            np.fill_diagonal(outm[:, t * P0:(t + 1) * P0], vals[:, t])
        return outm

    idx_s = np.concatenate([np.arange(64), np.arange(64)])
    dq0 = diag_pack(wq_d[0:128])
    dq1s = diag_pack(wq_d[128 + idx_s])
    dvA = diag_pack(wv_d[64:192])
    dv0s = diag_pack(wv_d[idx_s])

    wp_pack = np.zeros((P0, 384), f)
    WpT = w_proj.T.astype(f)
    wp_pack[:, 0:192] = WpT[0:P0]
    wp_pack[0:P1, 192:384] = WpT[P0:C]

    tqv = np.sqrt(np.repeat(temperature.reshape(HEADS).astype(f), CHD)).reshape(C, 1)

    shared = {
        "wk0": wk0.astype(h), "wk1": wk1.astype(h),
        "dq0": dq0.astype(h), "dq1s": dq1s.astype(h),
        "dvA": dvA.astype(h), "dv0s": dv0s.astype(h),
        "wp": wp_pack, "tq": tqv, "eye": np.eye(P0, dtype=h),
        "dvAc": np.ascontiguousarray(wv_d[64:192]).astype(h),
        "dv0c": np.ascontiguousarray(wv_d[idx_s]).astype(h),
    }
    maps = []
    for b in range(8):
        m = dict(shared)
        m["xb"] = np.ascontiguousarray(x[b].reshape(C, HW).astype(h))
        maps.append(m)
    return maps


def kernel(x, w_qkv, w_dw, w_proj, temperature, _trace=False, _iters=1):
    from concourse.bass_utils import run_bass_kernel_spmd
    if _iters not in _BUILT:
        _BUILT[_iters] = _build(_iters)
    nc = _BUILT[_iters]
    in_maps = _host_inputs(
        np.asarray(x), np.asarray(w_qkv), np.asarray(w_dw),
        np.asarray(w_proj), np.asarray(temperature))
    res = run_bass_kernel_spmd(nc, in_maps, list(range(8)), trace=_trace)
    outs = [res.results[i]["out"].astype(np.float32).reshape(C, H, W)
            for i in range(8)]
    y = np.stack(outs, axis=0)
    kernel.last_result = res
    return y
